# revision 7
# baseline (speedup 1.0000x reference)
"""Trainium2 Bass kernel for nn_MHA_2688649527670.

Reference computes, per batch b and head h:
    Q = x Wq_h^T, K = x Wk_h^T, V = x Wv_h^T          ([S, D] each)
    Z = softmax_over_d( (Q K^T / sqrt(D)) V )

No softmax between Q K^T and V, so the chain is associative:
    (Q K^T) V = x (Wq_h^T Wk_h G Wv_h^T) / sqrt(D),   G = x^T x   ([D, D])

which collapses the O(S^2 D) attention into a [D,D] weight chain plus one
[S,D]x[D,D*H] matmul, then softmax over d (free axis). Per-head per-row
softmax bias is mandatory (rowmax spread ~1500-2700 in logits).

Sharding: batch (4) x head-groups (2x4 heads) = 8 independent cores.

v3 notes (HW-measured, not cost-model):
  - x loaded ROW-BLOCK: partition p holds rows 16p..16p+15 (2KB/descriptor,
    4 dma_starts, BW-bound ~4.4us). G chunk n = rows {16p+n}; host reorders
    output (s = 16p + 2k + c).
  - front chain is semaphore-latency bound: bulk copies, split V/S halves.
    P0T in the middle of G (weights on SWDGE land ~10us).
  - xT transposes AFTER the chain emission, pair-0 first so finals start.
  - epilogue per pair: PE writes y f32 + bf16 shadow; V: MAX on shadow +
    SUM(bf16 t) + RECIP; S: 8 biased EXPs (2D [P,1] bias slices - 3D slices
    cost +110ns each); GpSimd: normalize MULT (bcast); out on sync queue.
"""

import ml_dtypes
import numpy as np

import concourse.bass as bass
import concourse.bacc as bacc
import concourse.mybir as mybir
import concourse.tile as tile
from concourse.bass_utils import run_bass_kernel_spmd
from concourse.masks import make_identity

B, S, D, H = 4, 2048, 128, 8
P = 128
HPC = H // 2          # heads per core
NCH = S // P          # 16 chunks; chunk n = rows {16p + n}
NPAIR = NCH // 2
N_CORES = 8
SCALE = 1.0 / float(np.sqrt(D))
F32 = mybir.dt.float32
F32R = mybir.dt.float32r
BF16 = mybir.dt.bfloat16

N_WARM = 3

_PROG = None


def _build_program():
    nc = bacc.Bacc("TRN2", target_bir_lowering=False, debug=False,
                   num_devices=N_CORES)

    x_d = nc.dram_tensor("x", [S, D], F32, kind="ExternalInput")
    wq_d = nc.dram_tensor("wq", [HPC * D, D], F32, kind="ExternalInput")
    wk_d = nc.dram_tensor("wk", [HPC * D, D], F32, kind="ExternalInput")
    wv_d = nc.dram_tensor("wv", [HPC * D, D], F32, kind="ExternalInput")
    # [pair, p, c, head, d] bf16; row s = 16p + 2*pair + c; host reorders
    out_d = nc.dram_tensor("out", [NPAIR, P, 2, HPC, D], BF16,
                           kind="ExternalOutput")

    with tile.TileContext(nc) as tc:
        with (
            tc.tile_pool(name="const", bufs=1) as const,
            tc.tile_pool(name="tpool", bufs=2) as tpool,
            tc.tile_pool(name="opool", bufs=3) as opool,
            tc.tile_pool(name="small", bufs=4) as small,
            tc.tile_pool(name="ps_pair", bufs=2, space="PSUM") as ps_pair,
            tc.tile_pool(name="ps_g", bufs=1, space="PSUM") as ps_g,
            tc.tile_pool(name="ps_t", bufs=2, space="PSUM") as ps_t,
        ):
            ident = const.tile([P, P], F32, tag="ident")
            make_identity(nc, ident)

            # ---- input DMAs first: x row-block on the 2 HWDGE queues ----
            x_sb = const.tile([P, NCH, D], F32, tag="x_sb")
            x_view = x_d.ap().rearrange("(p n) c -> p n c", p=P)
            for q in range(4):
                eng = nc.sync if q % 2 == 0 else nc.scalar
                eng.dma_start(x_sb[:, 4 * q:4 * q + 4, :],
                              x_view[:, 4 * q:4 * q + 4, :])
            w_sb = {}
            for nm, wd in (("wq", wq_d), ("wk", wk_d), ("wv", wv_d)):
                t = const.tile([P, HPC, D], F32, tag=f"{nm}_sb", name=f"{nm}_sb")
                nc.gpsimd.dma_start(t, wd.ap().rearrange("(h p) c -> p h c", p=P))
                w_sb[nm] = t

            # ---- PE p-state warmup (streak -> 2.4GHz) ----
            warm = const.tile([P, P], F32, tag="warm")
            nc.gpsimd.memset(warm, 0.0)
            g_ps = ps_g.tile([P, P], F32, tag="g_ps")
            for _ in range(N_WARM):
                nc.tensor.matmul(g_ps, lhsT=warm, rhs=warm, start=True,
                                 stop=True)

            # ---- G = x^T x; P0T mid-stream (weights land ~10us) ----
            p0t_pair = ps_pair.tile([P, 2, HPC * D], F32, tag="y")
            p0t_ps = p0t_pair[:, 0, :]
            for i in range(NCH):
                nc.tensor.matmul(g_ps, lhsT=x_sb[:, i, :], rhs=x_sb[:, i, :],
                                 start=(i == 0), stop=(i == NCH - 1))
                if i == 7:
                    for h in range(HPC):
                        nc.tensor.matmul(p0t_ps[:, h * D:(h + 1) * D],
                                         lhsT=w_sb["wk"][:, h, :],
                                         rhs=w_sb["wq"][:, h, :])
            # chain-critical: g copy (V), p0t scale-copy (S) in parallel
            g_sb = const.tile([P, P], F32R, tag="g_sb")
            nc.vector.tensor_copy(g_sb, g_ps)
            p0t_sb = const.tile([P, HPC * D], F32R, tag="p0t_sb")
            nc.scalar.mul(p0t_sb, p0t_ps, SCALE)

            wvt_pair = ps_pair.tile([P, 2, HPC * D], F32, tag="y")
            wvt_ps = wvt_pair[:, 0, :]
            for h in range(HPC):
                nc.tensor.transpose(wvt_ps[:, h * D:(h + 1) * D],
                                    w_sb["wv"][:, h, :], ident)
            wvt_sb = const.tile([P, HPC * D], F32R, tag="wvt_sb")
            nc.scalar.copy(wvt_sb, wvt_ps)

            # ---- UT = G @ P0T; M_h = UT_h^T WvT_h; bulk split copies ----
            ut_pair = ps_pair.tile([P, 2, HPC * D], F32, tag="y")
            ut_ps = ut_pair[:, 0, :]
            nc.tensor.matmul(ut_ps, lhsT=g_sb, rhs=p0t_sb)
            ut_sb = const.tile([P, HPC, D], F32R, tag="ut_sb")
            HW = HPC * D // 2
            nc.vector.tensor_copy(ut_sb[:, :2, :], ut_ps[:, :HW])
            nc.scalar.copy(ut_sb[:, 2:, :], ut_ps[:, HW:])
            m_pair = ps_pair.tile([P, 2, HPC * D], F32, tag="y")
            m_ps = m_pair[:, 0, :]
            for h in range(HPC):
                sl = slice(h * D, (h + 1) * D)
                nc.tensor.matmul(m_ps[:, sl], lhsT=ut_sb[:, h, :],
                                 rhs=wvt_sb[:, sl])
            m_all = const.tile([P, HPC * D], F32R, tag="m_all")
            nc.vector.tensor_copy(m_all[:, :HW], m_ps[:, :HW])
            nc.scalar.copy(m_all[:, HW:], m_ps[:, HW:])

            # ---- xT transposes: pair 0 first (gates finals), casts chase.
            #      4 transposes per ps_t bank, 2-chunk casts V/S alternate ----
            xT_sb = const.tile([P, NCH, D], F32R, tag="xT_sb")
            tp_banks = {}

            def emit_xt(i):
                b = i // 4
                if i % 4 == 0:
                    tp_banks[b] = ps_t.tile([P, 4, P], F32, tag="tp",
                                            name=f"tp{b}")
                nc.tensor.transpose(tp_banks[b][:, i % 4, :],
                                    x_sb[:, i, :], ident)

            def emit_cast2(j):  # chunks 2j, 2j+1
                b = j // 2
                src = tp_banks[b][:, 2 * (j % 2):2 * (j % 2) + 2, :]
                dst = xT_sb[:, 2 * j:2 * j + 2, :]
                if j % 2 == 0:
                    nc.vector.tensor_copy(dst, src)
                else:
                    nc.scalar.copy(dst, src)

            for i in range(NCH):
                emit_xt(i)
                if i % 2 == 1:
                    emit_cast2(i // 2)

            # ---- finals + pair-batched softmax epilogue ----
            live = {}

            def emit_front(k):
                y = ps_pair.tile([P, 2, HPC * D], F32, tag="y")
                for c in range(2):
                    nc.tensor.matmul(y[:, c, :], lhsT=xT_sb[:, 2 * k + c, :],
                                     rhs=m_all[:])
                negmax = small.tile([P, 2 * HPC], F32, tag="negmax")
                nc.vector.reduce_max(
                    out=negmax,
                    in_=y[:].rearrange("p c (h d) -> p (c h) d", h=HPC),
                    axis=mybir.AxisListType.X, negate=True)
                t_sb = tpool.tile([P, 2, HPC * D], BF16, tag="t_sb")
                for c in range(2):
                    for h in range(HPC):
                        i = c * HPC + h
                        nc.scalar.activation(
                            t_sb[:, c, h * D:(h + 1) * D],
                            y[:, c, h * D:(h + 1) * D],
                            mybir.ActivationFunctionType.Exp,
                            bias=negmax[:, i:i + 1], scale=1.0)
                live[k] = t_sb

            def emit_back(k):
                t_sb = live.pop(k)
                sums = small.tile([P, 2 * HPC], BF16, tag="sums")
                with nc.allow_low_precision(reason="bf16 exp sums, 2e-2 gate"):
                    nc.vector.reduce_sum(
                        out=sums,
                        in_=t_sb[:].rearrange("p c (h d) -> p (c h) d", h=HPC),
                        axis=mybir.AxisListType.X)
                rsum = small.tile([P, 2 * HPC], F32, tag="rsum")
                nc.vector.reciprocal(rsum, sums)
                o_sb = opool.tile([P, 2, HPC * D], BF16, tag="o_sb")
                rs_b = rsum.rearrange("p (c h) -> p c h", c=2)
                nc.gpsimd.tensor_tensor(
                    o_sb[:].rearrange("p c (h d) -> p c h d", h=HPC),
                    t_sb[:].rearrange("p c (h d) -> p c h d", h=HPC),
                    rs_b[:, :, :, None].to_broadcast((P, 2, HPC, D)),
                    mybir.AluOpType.mult)
                nc.sync.dma_start(out_d.ap()[k], o_sb)

            emit_front(0)
            for k in range(1, NPAIR):
                emit_front(k)
                emit_back(k - 1)
            emit_back(NPAIR - 1)

    nc.compile()
    return nc


def _get_program():
    global _PROG
    if _PROG is None:
        _PROG = _build_program()
    return _PROG


def _make_in_maps(x, W_q, W_k, W_v):
    in_maps = []
    for core in range(N_CORES):
        b, hg = core // 2, core % 2
        sl = slice(hg * HPC * D, (hg + 1) * HPC * D)
        in_maps.append({
            "x": np.ascontiguousarray(x[b]),
            "wq": np.ascontiguousarray(W_q[sl]),
            "wk": np.ascontiguousarray(W_k[sl]),
            "wv": np.ascontiguousarray(W_v[sl]),
        })
    return in_maps


def run(x, W_q, W_k, W_v, trace=False, **spmd_kwargs):
    """Run on 8 NeuronCores; returns (Z, BassKernelResults)."""
    nc = _get_program()
    in_maps = _make_in_maps(np.asarray(x, np.float32), np.asarray(W_q, np.float32),
                            np.asarray(W_k, np.float32), np.asarray(W_v, np.float32))
    res = run_bass_kernel_spmd(nc, in_maps, core_ids=list(range(N_CORES)),
                               trace=trace, **spmd_kwargs)
    Z = np.empty((B, H, S, D), np.float32)
    for core in range(N_CORES):
        b, hg = core // 2, core % 2
        o = np.asarray(res.results[core]["out"]).astype(np.float32)
        # [pair, p, c, (h d)] -> [h, s=16p+2*pair+c, d]
        o = o.reshape(NPAIR, P, 2, HPC, D)
        Z[b, hg * HPC:(hg + 1) * HPC] = (
            o.transpose(3, 1, 0, 2, 4).reshape(HPC, S, D))
    return Z, res


def kernel(x, W_q, W_k, W_v):
    Z, _ = run(x, W_q, W_k, W_v, trace=False)
    return Z


# revision 9
# speedup vs baseline: 1.2082x; 1.2082x over previous
"""Trainium2 Bass kernel for nn_MHA_2688649527670.

Reference computes, per batch b and head h:
    Q = x Wq_h^T, K = x Wk_h^T, V = x Wv_h^T          ([S, D] each)
    Z = softmax_over_d( (Q K^T / sqrt(D)) V )

No softmax between Q K^T and V, so the chain is associative:
    (Q K^T) V = x (Wq_h^T Wk_h G Wv_h^T) / sqrt(D),   G = x^T x   ([D, D])

which collapses the O(S^2 D) attention into a [D,D] weight chain plus one
[S,D]x[D,D*H] matmul, then softmax over d (free axis). Per-head softmax bias
is mandatory: per-head/row logit ranges span thousands.

Sharding: batch (4) x head-groups (2x4 heads) = 8 independent cores.

Perf notes (v4, HW-measured):
  - x loaded ROW-BLOCK: partition p holds rows 16p..16p+15, so each DMA
    descriptor is 2KB contiguous; 4 dma_starts land x ~6us earlier than the
    512B-line layout. G is chunk-order invariant (chunk n = rows {16p+n});
    host reorders the output (s = 16p + n).
  - G emitted as 4 accumulation sub-chains (start=False continuation) so the
    scheduler can interleave xT transposes / P0T between groups.
  - chain-critical copies (g, p0t, ut, m) run under tc.high_priority().
  - finals/UT in float32r (1 cyc/row at N=512); f32r written by producer.
  - epilogue per chunk (baseline-proven AP shapes, EXP=258ns needs 2D y
    and per-chunk tiles): V reduce_max -> 4x scalar Exp (per-head bias) ->
    V reduce_sum -> V reciprocal -> gpsimd normalize-mult (bf16 out) ->
    contiguous bf16 DMA out on the sync queue (host reorders/upcasts).
"""

import ml_dtypes
import numpy as np

import concourse.bass as bass
import concourse.bacc as bacc
import concourse.mybir as mybir
import concourse.tile as tile
from concourse.bass_utils import run_bass_kernel_spmd
from concourse.masks import make_identity

B, S, D, H = 4, 2048, 128, 8
P = 128
HPC = H // 2          # heads per core
NCH = S // P          # 16 chunks; chunk n = rows {16p + n}
N_CORES = 8
SCALE = 1.0 / float(np.sqrt(D))
F32 = mybir.dt.float32
F32R = mybir.dt.float32r
BF16 = mybir.dt.bfloat16

N_WARM = 4

_PROG = None


def _build_program():
    nc = bacc.Bacc("TRN2", target_bir_lowering=False, debug=False,
                   num_devices=N_CORES)

    x_d = nc.dram_tensor("x", [S, D], F32, kind="ExternalInput")
    wq_d = nc.dram_tensor("wq", [HPC * D, D], F32, kind="ExternalInput")
    wk_d = nc.dram_tensor("wk", [HPC * D, D], F32, kind="ExternalInput")
    wv_d = nc.dram_tensor("wv", [HPC * D, D], F32, kind="ExternalInput")
    # [chunk, p, head, d] bf16; row s = 16p + chunk; host reorders
    out_d = nc.dram_tensor("out", [NCH, P, HPC, D], BF16, kind="ExternalOutput")

    with tile.TileContext(nc) as tc:
        with (
            tc.tile_pool(name="const", bufs=1) as const,
            tc.tile_pool(name="work", bufs=6) as work,
            tc.tile_pool(name="small", bufs=4) as small,
            tc.tile_pool(name="ps_y", bufs=4, space="PSUM") as ps_y,
            tc.tile_pool(name="ps_g", bufs=1, space="PSUM") as ps_g,
            tc.tile_pool(name="ps_t", bufs=2, space="PSUM") as ps_t,
        ):
            ident = const.tile([P, P], F32, tag="ident")
            make_identity(nc, ident)

            # ---- input DMAs: x row-block on the two HW queues, weights on
            #      gpsimd SWDGE ----
            x_sb = const.tile([P, NCH, D], F32, tag="x_sb")
            x_view = x_d.ap().rearrange("(p n) c -> p n c", p=P)
            for q in range(4):
                eng = nc.sync if q % 2 == 0 else nc.scalar
                eng.dma_start(x_sb[:, 4 * q:4 * q + 4, :],
                              x_view[:, 4 * q:4 * q + 4, :])
            w_sb = {}
            for nm, wd in (("wq", wq_d), ("wk", wk_d), ("wv", wv_d)):
                t = const.tile([P, HPC, D], F32, tag=f"{nm}_sb", name=f"{nm}_sb")
                nc.gpsimd.dma_start(t, wd.ap().rearrange("(h p) c -> p h c", p=P))
                w_sb[nm] = t

            # ---- PE p-state warmup matmuls on a memset tile ----
            warm = const.tile([P, P], F32, tag="warm")
            nc.gpsimd.memset(warm, 0.0)
            g_ps = ps_g.tile([P, P], F32, tag="g_ps")
            for _ in range(N_WARM):
                nc.tensor.matmul(g_ps, lhsT=warm, rhs=warm, start=True,
                                 stop=True)

            # ---- G = x^T x as 4 sub-chains, xT transposes + P0T between ----
            xT_sb = const.tile([P, NCH, D], F32R, tag="xT_sb")
            tp_banks = {}

            def emit_xt(i):
                b = i // 4
                if i % 4 == 0:
                    tp_banks[b] = ps_t.tile([P, 4, P], F32, tag="tp",
                                            name=f"tp{b}")
                nc.tensor.transpose(tp_banks[b][:, i % 4, :],
                                    x_sb[:, i, :], ident)

            def emit_cast(b):
                if b % 2 == 0:
                    nc.vector.tensor_copy(xT_sb[:, 4 * b:4 * b + 4, :],
                                          tp_banks[b])
                else:
                    nc.scalar.copy(xT_sb[:, 4 * b:4 * b + 4, :], tp_banks[b])

            p0t_ps = ps_y.tile([P, HPC * D], F32, tag="c_ps")
            for grp in range(4):
                for i in range(4 * grp, 4 * grp + 4):
                    nc.tensor.matmul(g_ps, lhsT=x_sb[:, i, :],
                                     rhs=x_sb[:, i, :],
                                     start=(i == 0), stop=(i == NCH - 1),
                                     skip_group_check=(i != 0 and i % 4 == 0))
                # between groups: transposes of the landed quarter; P0T when
                # the weights have landed (~ group 2)
                for i in range(4 * grp, 4 * grp + 4):
                    emit_xt(i)
                if grp == 1:
                    emit_cast(0)
                if grp == 2:
                    for h in range(HPC):
                        nc.tensor.matmul(p0t_ps[:, h * D:(h + 1) * D],
                                         lhsT=w_sb["wk"][:, h, :],
                                         rhs=w_sb["wq"][:, h, :])
                    emit_cast(1)

            # chain-critical copies at high priority
            g_sb = const.tile([P, P], F32R, tag="g_sb")
            p0t_sb = const.tile([P, HPC * D], F32R, tag="p0t_sb")
            with tc.high_priority():
                nc.vector.tensor_copy(g_sb, g_ps)
                nc.scalar.mul(p0t_sb, p0t_ps, SCALE)

            wvt_ps = ps_y.tile([P, HPC * D], F32, tag="c_ps")
            for h in range(HPC):
                nc.tensor.transpose(wvt_ps[:, h * D:(h + 1) * D],
                                    w_sb["wv"][:, h, :], ident)
            wvt_sb = const.tile([P, HPC * D], F32R, tag="wvt_sb")
            nc.scalar.copy(wvt_sb, wvt_ps)
            emit_cast(2)
            emit_cast(3)

            # ---- UT = G @ P0T (G symmetric), one N=512 f32r matmul ----
            ut_ps = ps_y.tile([P, HPC * D], F32, tag="c_ps")
            ut_sb = const.tile([P, HPC * D], F32R, tag="ut_sb")
            m_ps = ps_y.tile([P, HPC * D], F32, tag="c_ps")
            m_all = const.tile([P, HPC * D], F32R, tag="m_all")
            HW = HPC * D // 2
            with tc.high_priority():
                nc.tensor.matmul(ut_ps, lhsT=g_sb, rhs=p0t_sb)
                nc.vector.tensor_copy(ut_sb[:, :HW], ut_ps[:, :HW])
                nc.scalar.copy(ut_sb[:, HW:], ut_ps[:, HW:])
                for h in range(HPC):
                    sl = slice(h * D, (h + 1) * D)
                    nc.tensor.matmul(m_ps[:, sl], lhsT=ut_sb[:, sl],
                                     rhs=wvt_sb[:, sl])
                nc.vector.tensor_copy(m_all[:, :HW], m_ps[:, :HW])
                nc.scalar.copy(m_all[:, HW:], m_ps[:, HW:])

            # ---- finals + software-pipelined softmax epilogue ----
            t_live = {}

            def emit_front(i):
                y_ps = ps_y.tile([P, HPC * D], F32, tag="c_ps")
                nc.tensor.matmul(y_ps, lhsT=xT_sb[:, i, :], rhs=m_all[:])
                negmax = small.tile([P, HPC], F32, tag="negmax")
                nc.vector.reduce_max(
                    out=negmax,
                    in_=y_ps[:].rearrange("p (h d) -> p h d", h=HPC),
                    axis=mybir.AxisListType.X, negate=True)
                t_sb = work.tile([P, HPC, D], F32, tag="t_sb")
                for h in range(HPC):
                    nc.scalar.activation(
                        t_sb[:, h, :], y_ps[:, h * D:(h + 1) * D],
                        mybir.ActivationFunctionType.Exp,
                        bias=negmax[:, h:h + 1], scale=1.0)
                t_live[i] = t_sb

            def emit_back(i):
                t_sb = t_live.pop(i)
                sums = small.tile([P, HPC], F32, tag="sums")
                nc.vector.reduce_sum(out=sums, in_=t_sb,
                                     axis=mybir.AxisListType.X)
                rsum = small.tile([P, HPC], F32, tag="rsum")
                nc.vector.reciprocal(rsum, sums)
                o_sb = work.tile([P, HPC, D], BF16, tag="o_sb")
                nc.gpsimd.tensor_tensor(
                    o_sb, t_sb,
                    rsum[:, :, None].to_broadcast((P, HPC, D)),
                    mybir.AluOpType.mult)
                nc.sync.dma_start(out_d.ap()[i], o_sb)

            emit_front(0)
            for i in range(1, NCH):
                emit_front(i)
                emit_back(i - 1)
            emit_back(NCH - 1)

    nc.compile()
    return nc


def _get_program():
    global _PROG
    if _PROG is None:
        _PROG = _build_program()
    return _PROG


def _make_in_maps(x, W_q, W_k, W_v):
    in_maps = []
    for core in range(N_CORES):
        b, hg = core // 2, core % 2
        sl = slice(hg * HPC * D, (hg + 1) * HPC * D)
        in_maps.append({
            "x": np.ascontiguousarray(x[b]),
            "wq": np.ascontiguousarray(W_q[sl]),
            "wk": np.ascontiguousarray(W_k[sl]),
            "wv": np.ascontiguousarray(W_v[sl]),
        })
    return in_maps


def run(x, W_q, W_k, W_v, trace=False, **spmd_kwargs):
    """Run on 8 NeuronCores; returns (Z, BassKernelResults)."""
    nc = _get_program()
    in_maps = _make_in_maps(np.asarray(x, np.float32), np.asarray(W_q, np.float32),
                            np.asarray(W_k, np.float32), np.asarray(W_v, np.float32))
    res = run_bass_kernel_spmd(nc, in_maps, core_ids=list(range(N_CORES)),
                               trace=trace, **spmd_kwargs)
    Z = np.empty((B, H, S, D), np.float32)
    for core in range(N_CORES):
        b, hg = core // 2, core % 2
        o = np.asarray(res.results[core]["out"]).astype(np.float32)
        # [chunk n, p, h, d] -> [h, s=16p+n, d]
        Z[b, hg * HPC:(hg + 1) * HPC] = (
            o.transpose(2, 1, 0, 3).reshape(HPC, S, D))
    return Z, res


def kernel(x, W_q, W_k, W_v):
    Z, _ = run(x, W_q, W_k, W_v, trace=False)
    return Z
